# revision 6
# baseline (speedup 1.0000x reference)
"""CompGCN layer kernel for 8 Trainium2 NeuronCores.

Strategy (dst-sharded, gather + selector-matmul aggregation, no collectives):
  - Each core owns 6250 destination nodes and receives exactly the edges
    whose dst falls in its range (host bucketing).
  - (h[src] + rel[type]) @ W == (h@W)[src] + (rel@W)[type]. Raw h rows are
    gathered and aggregated; W is applied once per dst-tile after summation.
  - Edges are grouped per dst-tile (128 dst nodes) into two statically-sized
    896-slot segments: A (src < 25088) and B (src >= 25088, local idx) —
    int16 gather indices can't span 50048 rows. GPSIMD dma_gather pulls
    h16[src] for GPR dst-tiles per op (the per-op fixed ~1-2us on the Pool
    sequencer dominates at small sizes; single_packet=False keeps packets
    within the 64-descriptor HW ceiling at large op sizes).
  - Aggregation runs in TRANSPOSED space: aggT[d, dst] += st16_chunk^T-as-
    stationary @ one-hot-selector (moving). This makes W/rel/loop all apply
    as natural matmuls with no on-chip transposes:
      psum2[d2, dst] = W^T-form @ (aggT*norm) + sum_rt relW_rt @ ctn_rt
                       + Wl @ hsT   ; out = relu(psum2)
  - norm is folded in once per dst-tile via a DVE mult (psum x normB), and
    into the count matrix ctn on the host for the rel term.
  - Everything loads as a few big contiguous DMAs; output accumulates in
    SBUF (transposed) and is written once at the end.
"""

import os
import numpy as np
from dataclasses import dataclass
from contextlib import ExitStack

from concourse import bacc, bass, mybir, tile
from concourse.bass_utils import run_bass_kernel_spmd

F32 = mybir.dt.float32
F16 = mybir.dt.float16
I16 = mybir.dt.int16

GPR = int(os.environ.get("KERNEL_GPR", "1"))  # dst-tiles per gather group


@dataclass(frozen=True)
class Cfg:
    n_nodes: int = 50000
    d: int = 128
    n_rels: int = 500
    n_cores: int = 8
    split: int = 25088
    n_edges: int = 600000
    na: int = 896          # per-tile A-segment slots (actual max 850)
    nb: int = 896          # per-tile B-segment slots (actual max 839)

    @property
    def np_core(self):
        return self.n_nodes // self.n_cores

    @property
    def np_pad(self):
        return ((self.np_core + 1 + 127) // 128) * 128

    @property
    def n_pad(self):
        return ((self.n_nodes + 127) // 128) * 128

    @property
    def r_pad(self):
        return ((self.n_rels + 127) // 128) * 128


CFG = Cfg()


def build_program(cfg: Cfg):
    nc = bacc.Bacc("TRN2", target_bir_lowering=False, debug=False,
                   num_swdge_queues=4,
                   dynamic_dma_scratch_size=16384 * GPR)
    D = cfg.d
    na, nb = cfg.na, cfg.nb
    seg = na + nb
    nt = cfg.np_pad // 128                  # 49 dst tiles
    n_r_tiles = cfg.r_pad // 128            # 4
    total_e = seg * nt
    ka = na // 128                          # 7 A-ktiles per dst tile
    ktg = seg // 128                        # 14 ktiles per dst tile
    groups = [min(GPR, nt - g) for g in range(0, nt, GPR)]

    h16 = nc.dram_tensor("h16", [cfg.n_pad, D], F16, kind="ExternalInput")
    rel_t = nc.dram_tensor("rel_t", [D, cfg.r_pad], F16, kind="ExternalInput")
    w16d = nc.dram_tensor("w16d", [D, D], F16, kind="ExternalInput")
    wl16d = nc.dram_tensor("wl16d", [D, D], F16, kind="ExternalInput")
    hs_t = nc.dram_tensor("hs_t", [D, cfg.np_pad], F16, kind="ExternalInput")
    normB = nc.dram_tensor("normB", [128, cfg.np_pad], F16, kind="ExternalInput")
    ct_r = nc.dram_tensor("ct_r", [128, n_r_tiles, nt, 128], F16, kind="ExternalInput")
    src_idx = nc.dram_tensor("src_idx", [128, total_e // 16], I16, kind="ExternalInput")
    dst_rel = nc.dram_tensor("dst_rel", [128, total_e // 128], F16, kind="ExternalInput")

    outT = nc.dram_tensor("outT", [128, nt * 128], F32, kind="ExternalOutput")

    with tile.TileContext(nc) as tc:
        with ExitStack() as ex:
            cpool = ex.enter_context(tc.tile_pool(name="const", bufs=1))
            inpool = ex.enter_context(tc.tile_pool(name="ld", bufs=2))
            pwpool = ex.enter_context(tc.tile_pool(name="psW", bufs=2, space="PSUM"))
            agpool = ex.enter_context(tc.tile_pool(name="psA", bufs=2, space="PSUM"))
            p2pool = ex.enter_context(tc.tile_pool(name="psF", bufs=2, space="PSUM"))
            segpool = ex.enter_context(tc.tile_pool(name="seg", bufs=2))
            selpool = ex.enter_context(tc.tile_pool(name="sel", bufs=2))
            fpool = ex.enter_context(tc.tile_pool(name="fin", bufs=3))

            # ------- constants / big contiguous loads -------
            iota_gb = cpool.tile([128, GPR * ktg, 128], F16)
            nc.gpsimd.iota(iota_gb[:], pattern=[[0, GPR * ktg], [1, 128]], base=0,
                           channel_multiplier=0,
                           allow_small_or_imprecise_dtypes=True)
            w16 = cpool.tile([128, D], F16)
            nc.sync.dma_start(out=w16[:], in_=w16d[:, :])
            wl16 = cpool.tile([128, D], F16)
            nc.sync.dma_start(out=wl16[:], in_=wl16d[:, :])
            hs_sb = cpool.tile([128, cfg.np_pad], F16)
            nc.sync.dma_start(out=hs_sb[:], in_=hs_t[:, :])
            nb_sb = cpool.tile([128, cfg.np_pad], F16)
            nc.sync.dma_start(out=nb_sb[:], in_=normB[:, :])
            ct_sb = cpool.tile([128, n_r_tiles, nt, 128], F16)
            nc.sync.dma_start(out=ct_sb[:], in_=ct_r[:, :, :, :])
            src_sb = cpool.tile([128, total_e // 16], I16)
            nc.sync.dma_start(out=src_sb[:], in_=src_idx[:, :])
            dc_sb = cpool.tile([128, total_e // 128], F16)
            nc.sync.dma_start(out=dc_sb[:], in_=dst_rel[:, :])
            outacc = cpool.tile([128, nt * 128], F32)

            relw16 = cpool.tile([128, n_r_tiles, D], F16)
            for rt in range(n_r_tiles):
                rtile = inpool.tile([128, D], F16, tag="rt16")
                nc.sync.dma_start(out=rtile[:], in_=rel_t[:, rt * D:(rt + 1) * D])
                pw = pwpool.tile([128, D], F32, tag="pw")
                nc.tensor.matmul(out=pw[:], lhsT=rtile[:], rhs=w16[:],
                                 start=True, stop=True)
                nc.vector.tensor_copy(out=relw16[:, rt, :], in_=pw[:])

            # ------- per group: 2 gathers; per dst-tile: matmuls + finish ----
            qctr = [0]

            def nextq():
                qctr[0] += 1
                return qctr[0] % 4

            for gi, gn in enumerate(groups):
                t0 = gi * GPR
                base = t0 * seg
                stg = segpool.tile([128, GPR * ktg, 128], F16, tag="seg")
                # A-block then B-block, each gn*896 idxs, one gather per block
                for blk, (part_len, tab) in enumerate(
                        ((na, h16[0:cfg.split, :]),
                         (nb, h16[cfg.split:cfg.n_pad, :]))):
                    n = gn * part_len
                    o = base + blk * gn * na
                    kb = blk * gn * ka
                    nc.gpsimd.dma_gather(
                        out_ap=stg[:, kb:kb + gn * ka, :],
                        in_ap=tab, idxs_ap=src_sb[:, o // 16:(o + n) // 16],
                        num_idxs=n, num_idxs_reg=n, elem_size=D,
                        queue_num=nextq())

                selg = selpool.tile([128, GPR * ktg, 128], F16, tag="sel")
                gw = gn * ktg
                dc_col = dc_sb[:, base // 128:base // 128 + gw]
                nc.vector.tensor_tensor(
                    out=selg[:, 0:gw, :], in0=iota_gb[:, 0:gw, :],
                    in1=dc_col.rearrange("p (c o) -> p c o", o=1).broadcast_to(
                        [128, gw, 128]),
                    op=mybir.AluOpType.is_equal)

                for j in range(gn):
                    t = t0 + j
                    aggT = agpool.tile([128, 128], F32, tag="aggT")
                    for k in range(ka):
                        nc.tensor.matmul(out=aggT[:],
                                         lhsT=stg[:, j * ka + k, :],
                                         rhs=selg[:, j * ka + k, :],
                                         start=(k == 0), stop=False)
                    boff = gn * ka
                    for k in range(ka):
                        nc.tensor.matmul(out=aggT[:],
                                         lhsT=stg[:, boff + j * ka + k, :],
                                         rhs=selg[:, boff + j * ka + k, :],
                                         start=False, stop=(k == ka - 1))
                    # fold norm while moving PSUM -> SBUF
                    aggTn = fpool.tile([128, 128], F16, tag="aggTn")
                    nc.vector.tensor_tensor(
                        out=aggTn[:], in0=aggT[:],
                        in1=nb_sb[:, t * 128:(t + 1) * 128],
                        op=mybir.AluOpType.mult)
                    # psum2 = W @ aggTn + sum_rt relW_rt @ ctn_rt + Wl @ hsT
                    p2 = p2pool.tile([128, 128], F32, tag="p2")
                    nc.tensor.matmul(out=p2[:], lhsT=w16[:], rhs=aggTn[:],
                                     start=True, stop=False)
                    for rt in range(n_r_tiles):
                        nc.tensor.matmul(out=p2[:], lhsT=relw16[:, rt, :],
                                         rhs=ct_sb[:, rt, t, :],
                                         start=False, stop=False)
                    nc.tensor.matmul(out=p2[:], lhsT=wl16[:],
                                     rhs=hs_sb[:, t * 128:(t + 1) * 128],
                                     start=False, stop=True)
                    nc.scalar.activation(out=outacc[:, t * 128:(t + 1) * 128],
                                         in_=p2[:],
                                         func=mybir.ActivationFunctionType.Relu)

            nc.sync.dma_start(out=outT[:, :], in_=outacc[:])

    nc.compile()
    return nc


def _wrap16(vals: np.ndarray, pad_len: int, pad_val: int) -> np.ndarray:
    a = np.full(pad_len, pad_val, dtype=np.int16)
    a[:len(vals)] = vals.astype(np.int16)
    w16 = a.reshape(pad_len // 16, 16).T
    return np.tile(w16, (8, 1)).copy()


def prep_inputs(cfg: Cfg, h, norm, rel_emb, w_nb, w_loop, edge_src, edge_dst, edge_type):
    h = np.asarray(h, np.float32)
    norm = np.asarray(norm, np.float32).reshape(-1)
    rel_emb = np.asarray(rel_emb, np.float32)
    edge_src = np.asarray(edge_src, np.int64)
    edge_dst = np.asarray(edge_dst, np.int64)
    edge_type = np.asarray(edge_type, np.int64)

    na, nb = cfg.na, cfg.nb
    seg = na + nb
    nt = cfg.np_pad // 128
    total_e = seg * nt
    groups = [min(GPR, nt - g) for g in range(0, nt, GPR)]

    h_pad = np.zeros((cfg.n_pad, cfg.d), np.float32)
    h_pad[:cfg.n_nodes] = h
    r_pad = np.zeros((cfg.r_pad, cfg.d), np.float32)
    r_pad[:cfg.n_rels] = rel_emb
    h16 = np.ascontiguousarray(h_pad.astype(np.float16))
    rel_t = np.ascontiguousarray(r_pad.T.astype(np.float16))
    w16d = np.asarray(w_nb, np.float16)
    wl16d = np.asarray(w_loop, np.float16)

    in_maps = []
    for c in range(cfg.n_cores):
        lo, hi = c * cfg.np_core, (c + 1) * cfg.np_core
        sel = (edge_dst >= lo) & (edge_dst < hi)
        src_c, dst_c, typ_c = edge_src[sel], edge_dst[sel] - lo, edge_type[sel]

        src_stream = np.zeros(total_e, np.int64)
        dc_stream = np.full(total_e, -1.0, np.float32)
        dtile = dst_c // 128
        in_a = src_c < cfg.split
        pos = 0
        for gi, gn in enumerate(groups):
            t0 = gi * GPR
            for blk, (plen, amask, soff) in enumerate(
                    ((na, in_a, 0), (nb, ~in_a, cfg.split))):
                for j in range(gn):
                    t = t0 + j
                    pm = (dtile == t) & amask
                    s = src_c[pm] - soff
                    d = dst_c[pm] - t * 128
                    assert len(s) <= plen, (c, t, blk, len(s), plen)
                    o = np.argsort(s, kind="stable")
                    src_stream[pos:pos + len(s)] = s[o]
                    dc_stream[pos:pos + len(s)] = d[o]
                    pos += plen
        assert pos == total_e

        hsl = np.zeros((cfg.np_pad, cfg.d), np.float32)
        hsl[:cfg.np_core] = h[lo:hi]
        hsl_t = np.ascontiguousarray(hsl.T.astype(np.float16))
        ntmp = np.zeros(cfg.np_pad, np.float32)
        ntmp[:cfg.np_core] = norm[lo:hi]
        normB = np.ascontiguousarray(
            np.broadcast_to(ntmp[None, :], (128, cfg.np_pad))).astype(np.float16)

        ct = np.zeros((cfg.r_pad, cfg.np_pad), np.float32)
        np.add.at(ct, (typ_c, dst_c), 1.0)
        ctn = (ct * ntmp[None, :]).astype(np.float16)
        # [r % 128, r // 128, dst_tile, dst % 128] contiguous per partition
        ct_r = np.ascontiguousarray(
            ctn.reshape(cfg.r_pad // 128, 128, nt, 128).transpose(1, 0, 2, 3))

        in_maps.append({
            "h16": h16, "rel_t": rel_t, "w16d": w16d, "wl16d": wl16d,
            "hs_t": hsl_t, "normB": normB, "ct_r": ct_r,
            "src_idx": _wrap16(src_stream, total_e, 0),
            "dst_rel": np.ascontiguousarray(
                dc_stream.reshape(total_e // 128, 128).T).astype(np.float16),
        })
    return in_maps


_CACHED = {}


def _get_program(cfg: Cfg):
    if cfg not in _CACHED:
        _CACHED[cfg] = build_program(cfg)
    return _CACHED[cfg]


LAST_RESULTS = None


def kernel(h, norm, rel_emb, W_neighbor, loop_weight, edge_src, edge_dst, edge_type):
    cfg = CFG
    nc = _get_program(cfg)
    in_maps = prep_inputs(cfg, h, norm, rel_emb, W_neighbor, loop_weight,
                          edge_src, edge_dst, edge_type)
    trace = os.environ.get("KERNEL_TRACE", "0") == "1"
    res = run_bass_kernel_spmd(nc, in_maps, list(range(cfg.n_cores)), trace=trace)
    global LAST_RESULTS
    LAST_RESULTS = res
    outs = [res.results[c]["outT"].reshape(128, cfg.np_pad).T[:cfg.np_core]
            for c in range(cfg.n_cores)]
    return np.concatenate(outs, axis=0).astype(np.float32)


# revision 7
# speedup vs baseline: 1.4899x; 1.4899x over previous
"""CompGCN layer kernel for 8 Trainium2 NeuronCores.

Strategy (dst-sharded, gather + selector-matmul aggregation, no collectives):
  - Each core owns 6250 destination nodes and receives exactly the edges
    whose dst falls in its range (host bucketing).
  - (h[src] + rel[type]) @ W == (h@W)[src] + (rel@W)[type]. Raw h rows are
    gathered and aggregated; W is applied once per dst-tile after summation.
  - Edges are grouped per dst-tile (128 dst nodes) into two statically-sized
    896-slot segments: A (src < 25088) and B (src >= 25088, local idx) —
    int16 gather indices can't span 50048 rows. GPSIMD dma_gather pulls
    h16[src] for GPR dst-tiles per op (the per-op fixed ~1-2us on the Pool
    sequencer dominates at small sizes; single_packet=False keeps packets
    within the 64-descriptor HW ceiling at large op sizes).
  - Aggregation runs in TRANSPOSED space: aggT[d, dst] += st16_chunk^T-as-
    stationary @ one-hot-selector (moving). This makes W/rel/loop all apply
    as natural matmuls with no on-chip transposes:
      psum2[d2, dst] = W^T-form @ (aggT*norm) + sum_rt relW_rt @ ctn_rt
                       + Wl @ hsT   ; out = relu(psum2)
  - norm is folded in once per dst-tile via a DVE mult (psum x normB), and
    into the count matrix ctn on the host for the rel term.
  - Everything loads as a few big contiguous DMAs; output accumulates in
    SBUF (transposed) and is written once at the end.
"""

import os
import numpy as np
from dataclasses import dataclass
from contextlib import ExitStack

from concourse import bacc, bass, mybir, tile
from concourse.bass_utils import run_bass_kernel_spmd

F32 = mybir.dt.float32
F16 = mybir.dt.float16
I16 = mybir.dt.int16

GPR = int(os.environ.get("KERNEL_GPR", "1"))  # dst-tiles per gather group


@dataclass(frozen=True)
class Cfg:
    n_nodes: int = 50000
    d: int = 128
    n_rels: int = 500
    n_cores: int = 8
    split: int = 25088
    n_edges: int = 600000
    na: int = 896          # per-tile A-segment slots (actual max 850)
    nb: int = 896          # per-tile B-segment slots (actual max 839)

    @property
    def np_core(self):
        return self.n_nodes // self.n_cores

    @property
    def np_pad(self):
        return ((self.np_core + 1 + 127) // 128) * 128

    @property
    def n_pad(self):
        return ((self.n_nodes + 127) // 128) * 128

    @property
    def r_pad(self):
        return ((self.n_rels + 127) // 128) * 128


CFG = Cfg()


def build_program(cfg: Cfg):
    nc = bacc.Bacc("TRN2", target_bir_lowering=False, debug=False,
                   num_swdge_queues=4,
                   dynamic_dma_scratch_size=16384 * GPR)
    D = cfg.d
    na, nb = cfg.na, cfg.nb
    seg = na + nb
    nt = cfg.np_pad // 128                  # 49 dst tiles
    n_r_tiles = cfg.r_pad // 128            # 4
    total_e = seg * nt
    ka = na // 128                          # 7 A-ktiles per dst tile
    ktg = seg // 128                        # 14 ktiles per dst tile
    groups = [min(GPR, nt - g) for g in range(0, nt, GPR)]

    h16 = nc.dram_tensor("h16", [cfg.n_pad, D], F16, kind="ExternalInput")
    rel_t = nc.dram_tensor("rel_t", [D, cfg.r_pad], F16, kind="ExternalInput")
    w16d = nc.dram_tensor("w16d", [D, D], F16, kind="ExternalInput")
    wl16d = nc.dram_tensor("wl16d", [D, D], F16, kind="ExternalInput")
    hs_t = nc.dram_tensor("hs_t", [D, cfg.np_pad], F16, kind="ExternalInput")
    normB = nc.dram_tensor("normB", [128, cfg.np_pad], F16, kind="ExternalInput")
    ct_r = nc.dram_tensor("ct_r", [128, n_r_tiles, nt, 128], F16, kind="ExternalInput")
    src_idx = nc.dram_tensor("src_idx", [128, total_e // 16], I16, kind="ExternalInput")
    dst_rel = nc.dram_tensor("dst_rel", [128, total_e // 128], F16, kind="ExternalInput")

    outT = nc.dram_tensor("outT", [128, nt * 128], F32, kind="ExternalOutput")

    with tile.TileContext(nc) as tc:
        with ExitStack() as ex:
            cpool = ex.enter_context(tc.tile_pool(name="const", bufs=1))
            inpool = ex.enter_context(tc.tile_pool(name="ld", bufs=2))
            pwpool = ex.enter_context(tc.tile_pool(name="psW", bufs=2, space="PSUM"))
            agpool = ex.enter_context(tc.tile_pool(name="psA", bufs=4, space="PSUM"))
            p2pool = ex.enter_context(tc.tile_pool(name="psF", bufs=2, space="PSUM"))
            segpool = ex.enter_context(tc.tile_pool(name="seg", bufs=6))
            selpool = ex.enter_context(tc.tile_pool(name="sel", bufs=6))
            fpool = ex.enter_context(tc.tile_pool(name="fin", bufs=6))

            # ------- constants / big contiguous loads -------
            iota_gb = cpool.tile([128, GPR * ktg, 128], F16)
            nc.gpsimd.iota(iota_gb[:], pattern=[[0, GPR * ktg], [1, 128]], base=0,
                           channel_multiplier=0,
                           allow_small_or_imprecise_dtypes=True)
            w16 = cpool.tile([128, D], F16)
            nc.sync.dma_start(out=w16[:], in_=w16d[:, :])
            wl16 = cpool.tile([128, D], F16)
            nc.sync.dma_start(out=wl16[:], in_=wl16d[:, :])
            hs_sb = cpool.tile([128, cfg.np_pad], F16)
            nc.sync.dma_start(out=hs_sb[:], in_=hs_t[:, :])
            nb_sb = cpool.tile([128, cfg.np_pad], F16)
            nc.sync.dma_start(out=nb_sb[:], in_=normB[:, :])
            ct_sb = cpool.tile([128, n_r_tiles, nt, 128], F16)
            nc.sync.dma_start(out=ct_sb[:], in_=ct_r[:, :, :, :])
            src_sb = cpool.tile([128, total_e // 16], I16)
            nc.sync.dma_start(out=src_sb[:], in_=src_idx[:, :])
            dc_sb = cpool.tile([128, total_e // 128], F16)
            nc.sync.dma_start(out=dc_sb[:], in_=dst_rel[:, :])
            outacc = cpool.tile([128, nt * 128], F32)

            relw16 = cpool.tile([128, n_r_tiles, D], F16)
            for rt in range(n_r_tiles):
                rtile = inpool.tile([128, D], F16, tag="rt16")
                nc.sync.dma_start(out=rtile[:], in_=rel_t[:, rt * D:(rt + 1) * D])
                pw = pwpool.tile([128, D], F32, tag="pw")
                nc.tensor.matmul(out=pw[:], lhsT=rtile[:], rhs=w16[:],
                                 start=True, stop=True)
                nc.vector.tensor_copy(out=relw16[:, rt, :], in_=pw[:])

            # ------- per group: 2 gathers; per dst-tile: matmuls + finish ----
            qctr = [0]

            def nextq():
                qctr[0] += 1
                return qctr[0] % 4

            for gi, gn in enumerate(groups):
                t0 = gi * GPR
                base = t0 * seg
                stg = segpool.tile([128, GPR * ktg, 128], F16, tag="seg")
                # A-block then B-block, each gn*896 idxs, one gather per block
                for blk, (part_len, tab) in enumerate(
                        ((na, h16[0:cfg.split, :]),
                         (nb, h16[cfg.split:cfg.n_pad, :]))):
                    n = gn * part_len
                    o = base + blk * gn * na
                    kb = blk * gn * ka
                    nc.gpsimd.dma_gather(
                        out_ap=stg[:, kb:kb + gn * ka, :],
                        in_ap=tab, idxs_ap=src_sb[:, o // 16:(o + n) // 16],
                        num_idxs=n, num_idxs_reg=n, elem_size=D,
                        queue_num=nextq())

                selg = selpool.tile([128, GPR * ktg, 128], F16, tag="sel")
                gw = gn * ktg
                dc_col = dc_sb[:, base // 128:base // 128 + gw]
                nc.vector.tensor_tensor(
                    out=selg[:, 0:gw, :], in0=iota_gb[:, 0:gw, :],
                    in1=dc_col.rearrange("p (c o) -> p c o", o=1).broadcast_to(
                        [128, gw, 128]),
                    op=mybir.AluOpType.is_equal)

                for j in range(gn):
                    t = t0 + j
                    aggT = agpool.tile([128, 128], F32, tag="aggT")
                    for k in range(ka):
                        nc.tensor.matmul(out=aggT[:],
                                         lhsT=stg[:, j * ka + k, :],
                                         rhs=selg[:, j * ka + k, :],
                                         start=(k == 0), stop=False)
                    boff = gn * ka
                    for k in range(ka):
                        nc.tensor.matmul(out=aggT[:],
                                         lhsT=stg[:, boff + j * ka + k, :],
                                         rhs=selg[:, boff + j * ka + k, :],
                                         start=False, stop=(k == ka - 1))
                    # fold norm while moving PSUM -> SBUF
                    aggTn = fpool.tile([128, 128], F16, tag="aggTn")
                    nc.vector.tensor_tensor(
                        out=aggTn[:], in0=aggT[:],
                        in1=nb_sb[:, t * 128:(t + 1) * 128],
                        op=mybir.AluOpType.mult)
                    # psum2 = W @ aggTn + sum_rt relW_rt @ ctn_rt + Wl @ hsT
                    p2 = p2pool.tile([128, 128], F32, tag="p2")
                    nc.tensor.matmul(out=p2[:], lhsT=w16[:], rhs=aggTn[:],
                                     start=True, stop=False)
                    for rt in range(n_r_tiles):
                        nc.tensor.matmul(out=p2[:], lhsT=relw16[:, rt, :],
                                         rhs=ct_sb[:, rt, t, :],
                                         start=False, stop=False)
                    nc.tensor.matmul(out=p2[:], lhsT=wl16[:],
                                     rhs=hs_sb[:, t * 128:(t + 1) * 128],
                                     start=False, stop=True)
                    nc.scalar.activation(out=outacc[:, t * 128:(t + 1) * 128],
                                         in_=p2[:],
                                         func=mybir.ActivationFunctionType.Relu)

            nc.sync.dma_start(out=outT[:, :], in_=outacc[:])

    nc.compile()
    return nc


def _wrap16(vals: np.ndarray, pad_len: int, pad_val: int) -> np.ndarray:
    a = np.full(pad_len, pad_val, dtype=np.int16)
    a[:len(vals)] = vals.astype(np.int16)
    w16 = a.reshape(pad_len // 16, 16).T
    return np.tile(w16, (8, 1)).copy()


def prep_inputs(cfg: Cfg, h, norm, rel_emb, w_nb, w_loop, edge_src, edge_dst, edge_type):
    h = np.asarray(h, np.float32)
    norm = np.asarray(norm, np.float32).reshape(-1)
    rel_emb = np.asarray(rel_emb, np.float32)
    edge_src = np.asarray(edge_src, np.int64)
    edge_dst = np.asarray(edge_dst, np.int64)
    edge_type = np.asarray(edge_type, np.int64)

    na, nb = cfg.na, cfg.nb
    seg = na + nb
    nt = cfg.np_pad // 128
    total_e = seg * nt
    groups = [min(GPR, nt - g) for g in range(0, nt, GPR)]

    h_pad = np.zeros((cfg.n_pad, cfg.d), np.float32)
    h_pad[:cfg.n_nodes] = h
    r_pad = np.zeros((cfg.r_pad, cfg.d), np.float32)
    r_pad[:cfg.n_rels] = rel_emb
    h16 = np.ascontiguousarray(h_pad.astype(np.float16))
    rel_t = np.ascontiguousarray(r_pad.T.astype(np.float16))
    w16d = np.asarray(w_nb, np.float16)
    wl16d = np.asarray(w_loop, np.float16)

    in_maps = []
    for c in range(cfg.n_cores):
        lo, hi = c * cfg.np_core, (c + 1) * cfg.np_core
        sel = (edge_dst >= lo) & (edge_dst < hi)
        src_c, dst_c, typ_c = edge_src[sel], edge_dst[sel] - lo, edge_type[sel]

        src_stream = np.zeros(total_e, np.int64)
        dc_stream = np.full(total_e, -1.0, np.float32)
        dtile = dst_c // 128
        in_a = src_c < cfg.split
        pos = 0
        for gi, gn in enumerate(groups):
            t0 = gi * GPR
            for blk, (plen, amask, soff) in enumerate(
                    ((na, in_a, 0), (nb, ~in_a, cfg.split))):
                for j in range(gn):
                    t = t0 + j
                    pm = (dtile == t) & amask
                    s = src_c[pm] - soff
                    d = dst_c[pm] - t * 128
                    assert len(s) <= plen, (c, t, blk, len(s), plen)
                    o = np.argsort(s, kind="stable")
                    src_stream[pos:pos + len(s)] = s[o]
                    dc_stream[pos:pos + len(s)] = d[o]
                    pos += plen
        assert pos == total_e

        hsl = np.zeros((cfg.np_pad, cfg.d), np.float32)
        hsl[:cfg.np_core] = h[lo:hi]
        hsl_t = np.ascontiguousarray(hsl.T.astype(np.float16))
        ntmp = np.zeros(cfg.np_pad, np.float32)
        ntmp[:cfg.np_core] = norm[lo:hi]
        normB = np.ascontiguousarray(
            np.broadcast_to(ntmp[None, :], (128, cfg.np_pad))).astype(np.float16)

        ct = np.zeros((cfg.r_pad, cfg.np_pad), np.float32)
        np.add.at(ct, (typ_c, dst_c), 1.0)
        ctn = (ct * ntmp[None, :]).astype(np.float16)
        # [r % 128, r // 128, dst_tile, dst % 128] contiguous per partition
        ct_r = np.ascontiguousarray(
            ctn.reshape(cfg.r_pad // 128, 128, nt, 128).transpose(1, 0, 2, 3))

        in_maps.append({
            "h16": h16, "rel_t": rel_t, "w16d": w16d, "wl16d": wl16d,
            "hs_t": hsl_t, "normB": normB, "ct_r": ct_r,
            "src_idx": _wrap16(src_stream, total_e, 0),
            "dst_rel": np.ascontiguousarray(
                dc_stream.reshape(total_e // 128, 128).T).astype(np.float16),
        })
    return in_maps


_CACHED = {}


def _get_program(cfg: Cfg):
    if cfg not in _CACHED:
        _CACHED[cfg] = build_program(cfg)
    return _CACHED[cfg]


LAST_RESULTS = None


def kernel(h, norm, rel_emb, W_neighbor, loop_weight, edge_src, edge_dst, edge_type):
    cfg = CFG
    nc = _get_program(cfg)
    in_maps = prep_inputs(cfg, h, norm, rel_emb, W_neighbor, loop_weight,
                          edge_src, edge_dst, edge_type)
    trace = os.environ.get("KERNEL_TRACE", "0") == "1"
    res = run_bass_kernel_spmd(nc, in_maps, list(range(cfg.n_cores)), trace=trace)
    global LAST_RESULTS
    LAST_RESULTS = res
    outs = [res.results[c]["outT"].reshape(128, cfg.np_pad).T[:cfg.np_core]
            for c in range(cfg.n_cores)]
    return np.concatenate(outs, axis=0).astype(np.float32)
